# revision 13
# baseline (speedup 1.0000x reference)
"""MoE block kernel for Trainium2 (8 NeuronCores, Bass/Tile).

Strategy: expert-parallel with host-side top-2 dispatch.
  - Host computes the gate (softmax + top-2) in numpy (0.01% of FLOPs) and
    gathers each expert's tokens into a padded, transposed buffer.
  - Core e runs expert e's FFN over its gathered tokens (capacity C) plus
    the shared-expert FFN over a 1024-token slice, all in fp32r matmuls
    (full-rate PE, ~1.4e-4 matmul rel err) with fp32 PSUM accumulation.
  - Routing weights are folded into the activations after GELU (scale
    commutes with the second matmul); biases enter phase B as a K=1
    matmul row so outputs need no postprocessing.
  - Host scatters expert outputs back (y[idx_e] += ...) and stitches the
    shared slices.

Layouts (per core):
  phase A: g[i_tile] [128(I), tok]  = GELU(w1T_tile.T @ xT + b1) * wscale
  phase B: yT[d_tile] [128(D), tok] = sum_i w2T_tile.T @ g[i] + b2*wscale
"""

import os

import numpy as np

B, S, D, E, I = 2, 4096, 1024, 8, 4096
T = B * S
TOP_K = 2
TS = T // 8          # shared-expert tokens per core
CHUNK = 512          # tokens per weight-stream pass (one N=512 matmul per LDW)
P = 128

LAST_RESULTS = None  # BassKernelResults of the most recent run (set when tracing)


def _round_up(x, m):
    return ((x + m - 1) // m) * m


def _chunks(total):
    """Split total tokens into chunks of CHUNK then a 256-multiple tail."""
    out = []
    t = 0
    while total - t >= CHUNK:
        out.append((t, CHUNK))
        t += CHUNK
    if total - t:
        out.append((t, total - t))
        t = total
    return out


def _pieces(n):
    """Split a chunk into matmul moving-dim pieces of at most 512."""
    out = []
    t = 0
    while n - t > 512:
        out.append((t, 512))
        t += 512
    out.append((t, n - t))
    return out


def _build_program(C):
    import concourse.mybir as mybir
    import concourse.tile as tile
    from concourse import bacc

    F32, F32R = mybir.dt.float32, mybir.dt.float32r
    AF = mybir.ActivationFunctionType

    nc = bacc.Bacc("TRN2", target_bir_lowering=False, debug=False)

    xgT_d = nc.dram_tensor("xgT", [D, C], F32R, kind="ExternalInput")
    wscb_d = nc.dram_tensor("wscb", [P, C], F32R, kind="ExternalInput")
    w1T_d = nc.dram_tensor("w1T", [D, I], F32R, kind="ExternalInput")
    b1_d = nc.dram_tensor("b1", [I], F32, kind="ExternalInput")
    w2T_d = nc.dram_tensor("w2T", [I, D], F32R, kind="ExternalInput")
    b2_d = nc.dram_tensor("b2", [D], F32, kind="ExternalInput")
    xsT_d = nc.dram_tensor("xsT", [D, TS], F32R, kind="ExternalInput")
    sw1T_d = nc.dram_tensor("sw1T", [D, I], F32R, kind="ExternalInput")
    sb1_d = nc.dram_tensor("sb1", [I], F32, kind="ExternalInput")
    sw2T_d = nc.dram_tensor("sw2T", [I, D], F32R, kind="ExternalInput")
    sb2_d = nc.dram_tensor("sb2", [D], F32, kind="ExternalInput")
    yeT_d = nc.dram_tensor("yeT", [D, C], F32, kind="ExternalOutput")
    ysT_d = nc.dram_tensor("ysT", [D, TS], F32, kind="ExternalOutput")

    DT, IT = D // P, I // P        # 8 d-subtiles, 32 i-tiles
    IG = 8                         # i-tiles per w2 stream group
    MAXCH = CHUNK

    with tile.TileContext(nc) as tc:
        with (
            tc.tile_pool(name="const", bufs=1) as const,
            tc.tile_pool(name="act", bufs=1) as act,
            tc.tile_pool(name="xin", bufs=2) as xin,
            tc.tile_pool(name="w1p", bufs=8) as w1p,
            tc.tile_pool(name="w2p", bufs=8) as w2p,
            tc.tile_pool(name="ev", bufs=4) as ev,
            tc.tile_pool(name="psA", bufs=2, space="PSUM") as psA,
            tc.tile_pool(name="psB", bufs=2, space="PSUM") as psB,
        ):
            b1t = const.tile([P, IT], F32, tag="b1")
            nc.sync.dma_start(b1t[:], b1_d.ap().rearrange("(o p) -> p o", p=P))
            sb1t = const.tile([P, IT], F32, tag="sb1")
            nc.sync.dma_start(sb1t[:], sb1_d.ap().rearrange("(o p) -> p o", p=P))
            b2t = const.tile([P, DT], F32, tag="b2")
            nc.sync.dma_start(b2t[:], b2_d.ap().rearrange("(o p) -> p o", p=P))
            sb2t = const.tile([P, DT], F32, tag="sb2")
            nc.sync.dma_start(sb2t[:], sb2_d.ap().rearrange("(o p) -> p o", p=P))
            wscb = const.tile([P, C], F32R, tag="wscb")
            nc.sync.dma_start(wscb[:], wscb_d.ap())

            g = act.tile([P, IT, MAXCH], F32R, tag="g")

            def ffn(xT_dram, ntok, w1T_dram, b1_tile, w2T_dram, b2_tile,
                    sc_bcast, outT_dram):
                w1r = w1T_dram.ap().rearrange("(o p) i -> p o i", p=P)
                w2r = w2T_dram.ap().rearrange("(o p) d -> p o d", p=P)
                xr = xT_dram.ap().rearrange("(o p) t -> p o t", p=P)
                outr = outT_dram.ap().rearrange("(o p) t -> p o t", p=P)
                for c0, cn in _chunks(ntok):
                    pieces = _pieces(cn)
                    xt = xin.tile([P, DT, MAXCH], F32R, tag="x")
                    nc.gpsimd.dma_start(xt[:, :, :cn], xr[:, :, c0:c0 + cn])
                    # Phase A: g[i] = gelu(w1T_i.T @ x + b1_i) (* wscale)
                    # w1 streams on the Scalar HWDGE ring, w2 on the Sync ring
                    # so the two weight streams issue in parallel.
                    for i in range(IT):
                        w1t = w1p.tile([P, DT, P], F32R, tag="w1")
                        nc.scalar.dma_start(w1t[:], w1r[:, :, i * P:(i + 1) * P])
                        pa = psA.tile([P, MAXCH], F32, tag="psA")
                        for k in range(DT):
                            for p0, pn in pieces:
                                nc.tensor.matmul(
                                    pa[:, p0:p0 + pn], w1t[:, k],
                                    xt[:, k, p0:p0 + pn],
                                    start=(k == 0), stop=(k == DT - 1))
                        nc.scalar.activation(g[:, i, :cn], pa[:, :cn], AF.Gelu,
                                             bias=b1_tile[:, i, None])
                        if sc_bcast is not None:
                            nc.vector.tensor_mul(
                                out=g[:, i, :cn], in0=g[:, i, :cn],
                                in1=sc_bcast[:, c0:c0 + cn])
                    # Phase B: yT[d] = sum_i w2T_(i,d).T @ g[i] + b2_d * sc
                    for d in range(DT):
                        pb = psB.tile([P, MAXCH], F32, tag="psB")
                        for ig in range(IT // IG):
                            w2t = w2p.tile([P, IG, P], F32R, tag="w2")
                            nc.sync.dma_start(
                                w2t[:], w2r[:, ig * IG:(ig + 1) * IG,
                                            d * P:(d + 1) * P])
                            for ii in range(IG):
                                i = ig * IG + ii
                                for p0, pn in pieces:
                                    nc.tensor.matmul(
                                        pb[:, p0:p0 + pn], w2t[:, ii],
                                        g[:, i, p0:p0 + pn],
                                        start=(i == 0), stop=(i == IT - 1))
                        yt = ev.tile([P, MAXCH], F32, tag="ev")
                        if sc_bcast is not None:
                            # (f + b2) * w == f*w + b2*w; f*w is already in
                            # psum (g was pre-scaled), add b2[d]*wscale here.
                            bw = ev.tile([P, MAXCH], F32, tag="bw")
                            nc.vector.tensor_scalar_mul(
                                bw[:, :cn], sc_bcast[:, c0:c0 + cn],
                                b2_tile[:, d, None])
                            nc.vector.tensor_add(
                                out=yt[:, :cn], in0=pb[:, :cn], in1=bw[:, :cn])
                        else:
                            nc.vector.tensor_scalar_add(
                                yt[:, :cn], pb[:, :cn], b2_tile[:, d, None])
                        nc.gpsimd.dma_start(outr[:, d, c0:c0 + cn], yt[:, :cn])

            ffn(xgT_d, C, w1T_d, b1t, w2T_d, b2t, wscb, yeT_d)
            ffn(xsT_d, TS, sw1T_d, sb1t, sw2T_d, sb2t, None, ysT_d)

    nc.compile()
    return nc


_PROGRAM_CACHE = {}


def _get_program(C):
    if C not in _PROGRAM_CACHE:
        _PROGRAM_CACHE[C] = _build_program(C)
    return _PROGRAM_CACHE[C]


def _install_trace_shim():
    """Provide antenv.axon_hooks so run_bass_kernel_spmd(trace=True) can
    capture NTFF profiles under axon (mirrors trn_agent_boot.trn_boot)."""
    import contextlib
    import ctypes
    import sys
    import types

    if "antenv.axon_hooks" in sys.modules:
        return
    so_path = "/opt/axon/libaxon_pjrt.so"
    hook = None
    try:
        lib = ctypes.CDLL(so_path)
        if hasattr(lib, "axon_start_nrt_profile"):
            lib.axon_start_nrt_profile.argtypes = [
                ctypes.POINTER(ctypes.c_int64), ctypes.c_size_t]
            lib.axon_start_nrt_profile.restype = ctypes.c_int64
            lib.axon_stop_nrt_profile.argtypes = [ctypes.c_char_p]
            lib.axon_stop_nrt_profile.restype = ctypes.c_int64

            @contextlib.contextmanager
            def _hook(output_dir, device_ids):
                import jax
                jax.devices()
                if device_ids:
                    ids = (ctypes.c_int64 * len(device_ids))(*device_ids)
                    rc = lib.axon_start_nrt_profile(ids, len(device_ids))
                else:
                    rc = lib.axon_start_nrt_profile(None, 0)
                if rc != 0:
                    raise RuntimeError(f"axon_start_nrt_profile rc={rc}")
                try:
                    yield
                finally:
                    n = lib.axon_stop_nrt_profile(str(output_dir).encode())
                    print(f"ntff profile: {n} file(s) -> {output_dir}",
                          file=sys.stderr)

            hook = _hook
    except OSError:
        pass
    mod = types.ModuleType("antenv.axon_hooks")
    mod.get_axon_ntff_profile_hook = lambda: hook
    mod.set_axon_ntff_profile_hook = lambda h: None
    sys.modules["antenv.axon_hooks"] = mod
    import antenv
    antenv.axon_hooks = mod


def kernel(hidden_states, gate_w, e_w1, e_b1, e_w2, e_b2,
           s_w1, s_b1, s_w2, s_b2):
    global LAST_RESULTS
    from concourse.bass_utils import run_bass_kernel_spmd

    hidden_states = np.asarray(hidden_states, dtype=np.float32)
    gate_w = np.asarray(gate_w, dtype=np.float32)
    x = np.ascontiguousarray(hidden_states.reshape(T, D))

    # ---- gate: softmax + top-2 (host; 0.01% of total FLOPs) ----
    # float64 so the ranking agrees with any fp32 reference implementation
    # (fp32 impls deviate ~1e-8 from f64 in prob space; min top2/top3 margin
    # on this distribution is ~5e-7).
    logits = x.astype(np.float64) @ gate_w.T.astype(np.float64)
    m = logits.max(axis=-1, keepdims=True)
    p = np.exp(logits - m)
    p /= p.sum(axis=-1, keepdims=True)
    order = np.argsort(-p, axis=-1, kind="stable")
    top_idx = order[:, :TOP_K]                       # [T, 2]
    top_w = np.take_along_axis(p, top_idx, axis=-1)  # [T, 2]

    idx_e = [np.where((top_idx == e).any(axis=1))[0] for e in range(E)]
    w_e = []
    for e in range(E):
        sel = top_idx[idx_e[e]] == e
        w_e.append((top_w[idx_e[e]] * sel).sum(axis=1).astype(np.float32))

    counts = np.array([len(ix) for ix in idx_e])
    C = max(CHUNK, _round_up(int(counts.max()), CHUNK))

    nc = _get_program(C)

    xT = np.ascontiguousarray(x.T)  # [D, T]
    in_maps = []
    for e in range(E):
        n_e = counts[e]
        xgT = np.zeros((D, C), np.float32)
        xgT[:, :n_e] = xT[:, idx_e[e]]
        wsc = np.zeros((C,), np.float32)
        wsc[:n_e] = w_e[e]
        in_maps.append({
            "xgT": xgT,
            "wscb": np.broadcast_to(wsc, (P, C)).copy(),
            "w1T": np.ascontiguousarray(e_w1[e].T.astype(np.float32)),
            "b1": np.ascontiguousarray(e_b1[e].astype(np.float32)),
            "w2T": np.ascontiguousarray(e_w2[e].T.astype(np.float32)),
            "b2": np.ascontiguousarray(e_b2[e].astype(np.float32)),
            "xsT": np.ascontiguousarray(xT[:, e * TS:(e + 1) * TS]),
            "sw1T": np.ascontiguousarray(np.asarray(s_w1, np.float32).T),
            "sb1": np.ascontiguousarray(np.asarray(s_b1, np.float32)),
            "sw2T": np.ascontiguousarray(np.asarray(s_w2, np.float32).T),
            "sb2": np.ascontiguousarray(np.asarray(s_b2, np.float32)),
        })

    trace = os.environ.get("MOE_TRACE", "0") == "1"
    kwargs = {}
    if trace:
        _install_trace_shim()
        kwargs = dict(trace=True,
                      tmpdir=os.environ.get("MOE_TRACE_DIR") or None)
    res = run_bass_kernel_spmd(nc, in_maps, core_ids=list(range(E)), **kwargs)
    LAST_RESULTS = res

    y = np.empty((T, D), np.float32)
    for e in range(E):
        y[e * TS:(e + 1) * TS] = res.results[e]["ysT"].T
    for e in range(E):
        n_e = counts[e]
        y[idx_e[e]] += res.results[e]["yeT"][:, :n_e].T
    return y.reshape(B, S, D)


# revision 14
# speedup vs baseline: 1.0409x; 1.0409x over previous
"""MoE block kernel for Trainium2 (8 NeuronCores, Bass/Tile).

Strategy: expert-parallel with host-side top-2 dispatch.
  - Host computes the gate (softmax + top-2) in numpy (0.01% of FLOPs) and
    gathers each expert's tokens into a padded, transposed buffer.
  - Core e runs expert e's FFN over its gathered tokens (capacity C) plus
    the shared-expert FFN over a 1024-token slice, all in fp32r matmuls
    (full-rate PE, ~1.4e-4 matmul rel err) with fp32 PSUM accumulation.
  - Routing weights are folded into the activations after GELU (scale
    commutes with the second matmul); biases enter phase B as a K=1
    matmul row so outputs need no postprocessing.
  - Host scatters expert outputs back (y[idx_e] += ...) and stitches the
    shared slices.

Layouts (per core):
  phase A: g[i_tile] [128(I), tok]  = GELU(w1T_tile.T @ xT + b1) * wscale
  phase B: yT[d_tile] [128(D), tok] = sum_i w2T_tile.T @ g[i] + b2*wscale
"""

import os

import numpy as np

B, S, D, E, I = 2, 4096, 1024, 8, 4096
T = B * S
TOP_K = 2
TS = T // 8          # shared-expert tokens per core
CHUNK = 512          # tokens per weight-stream pass (one N=512 matmul per LDW)
P = 128

LAST_RESULTS = None  # BassKernelResults of the most recent run (set when tracing)


def _round_up(x, m):
    return ((x + m - 1) // m) * m


def _chunks(total):
    """Split total tokens into chunks of CHUNK then a 256-multiple tail."""
    out = []
    t = 0
    while total - t >= CHUNK:
        out.append((t, CHUNK))
        t += CHUNK
    if total - t:
        out.append((t, total - t))
        t = total
    return out


def _pieces(n):
    """Split a chunk into matmul moving-dim pieces of at most 512."""
    out = []
    t = 0
    while n - t > 512:
        out.append((t, 512))
        t += 512
    out.append((t, n - t))
    return out


def _build_program(C):
    import concourse.mybir as mybir
    import concourse.tile as tile
    from concourse import bacc

    F32, F32R = mybir.dt.float32, mybir.dt.float32r
    AF = mybir.ActivationFunctionType

    nc = bacc.Bacc("TRN2", target_bir_lowering=False, debug=False)

    xgT_d = nc.dram_tensor("xgT", [D, C], F32R, kind="ExternalInput")
    wscb_d = nc.dram_tensor("wscb", [P, C], F32R, kind="ExternalInput")
    w1T_d = nc.dram_tensor("w1T", [D, I], F32R, kind="ExternalInput")
    b1_d = nc.dram_tensor("b1", [I], F32, kind="ExternalInput")
    w2T_d = nc.dram_tensor("w2T", [I, D], F32R, kind="ExternalInput")
    b2_d = nc.dram_tensor("b2", [D], F32, kind="ExternalInput")
    xsT_d = nc.dram_tensor("xsT", [D, TS], F32R, kind="ExternalInput")
    sw1T_d = nc.dram_tensor("sw1T", [D, I], F32R, kind="ExternalInput")
    sb1_d = nc.dram_tensor("sb1", [I], F32, kind="ExternalInput")
    sw2T_d = nc.dram_tensor("sw2T", [I, D], F32R, kind="ExternalInput")
    sb2_d = nc.dram_tensor("sb2", [D], F32, kind="ExternalInput")
    yeT_d = nc.dram_tensor("yeT", [D, C], F32, kind="ExternalOutput")
    ysT_d = nc.dram_tensor("ysT", [D, TS], F32, kind="ExternalOutput")

    DT, IT = D // P, I // P        # 8 d-subtiles, 32 i-tiles
    IG = 8                         # i-tiles per w2 stream group
    MAXCH = CHUNK

    with tile.TileContext(nc) as tc:
        with (
            tc.tile_pool(name="const", bufs=1) as const,
            tc.tile_pool(name="act", bufs=1) as act,
            tc.tile_pool(name="xin", bufs=2) as xin,
            tc.tile_pool(name="w1p", bufs=8) as w1p,
            tc.tile_pool(name="w2p", bufs=8) as w2p,
            tc.tile_pool(name="ev", bufs=4) as ev,
            tc.tile_pool(name="psA", bufs=2, space="PSUM") as psA,
            tc.tile_pool(name="psB", bufs=2, space="PSUM") as psB,
        ):
            b1t = const.tile([P, IT], F32, tag="b1")
            nc.sync.dma_start(b1t[:], b1_d.ap().rearrange("(o p) -> p o", p=P))
            sb1t = const.tile([P, IT], F32, tag="sb1")
            nc.sync.dma_start(sb1t[:], sb1_d.ap().rearrange("(o p) -> p o", p=P))
            b2t = const.tile([P, DT], F32, tag="b2")
            nc.sync.dma_start(b2t[:], b2_d.ap().rearrange("(o p) -> p o", p=P))
            sb2t = const.tile([P, DT], F32, tag="sb2")
            nc.sync.dma_start(sb2t[:], sb2_d.ap().rearrange("(o p) -> p o", p=P))
            wscb = const.tile([P, C], F32R, tag="wscb")
            nc.gpsimd.dma_start(wscb[:], wscb_d.ap())

            g = act.tile([P, IT, MAXCH], F32R, tag="g")

            def ffn(xT_dram, ntok, w1T_dram, b1_tile, w2T_dram, b2_tile,
                    sc_bcast, outT_dram):
                w1r = w1T_dram.ap().rearrange("(o p) i -> p o i", p=P)
                w2r = w2T_dram.ap().rearrange("(o p) d -> p o d", p=P)
                xr = xT_dram.ap().rearrange("(o p) t -> p o t", p=P)
                outr = outT_dram.ap().rearrange("(o p) t -> p o t", p=P)
                for c0, cn in _chunks(ntok):
                    pieces = _pieces(cn)
                    xt = xin.tile([P, DT, MAXCH], F32R, tag="x")
                    nc.sync.dma_start(xt[:, :, :cn], xr[:, :, c0:c0 + cn])
                    # Phase A: g[i] = gelu(w1T_i.T @ x + b1_i) (* wscale)
                    # w1+x stream on the Sync HWDGE ring, w2 on the Scalar
                    # ring, outputs via SWDGE: three parallel DMA paths.
                    for i in range(IT):
                        w1t = w1p.tile([P, DT, P], F32R, tag="w1")
                        nc.sync.dma_start(w1t[:], w1r[:, :, i * P:(i + 1) * P])
                        pa = psA.tile([P, MAXCH], F32, tag="psA")
                        for k in range(DT):
                            for p0, pn in pieces:
                                nc.tensor.matmul(
                                    pa[:, p0:p0 + pn], w1t[:, k],
                                    xt[:, k, p0:p0 + pn],
                                    start=(k == 0), stop=(k == DT - 1))
                        nc.scalar.activation(g[:, i, :cn], pa[:, :cn], AF.Gelu,
                                             bias=b1_tile[:, i, None])
                        if sc_bcast is not None:
                            nc.vector.tensor_mul(
                                out=g[:, i, :cn], in0=g[:, i, :cn],
                                in1=sc_bcast[:, c0:c0 + cn])
                    # Phase B: yT[d] = sum_i w2T_(i,d).T @ g[i] + b2_d * sc
                    for d in range(DT):
                        pb = psB.tile([P, MAXCH], F32, tag="psB")
                        for ig in range(IT // IG):
                            w2t = w2p.tile([P, IG, P], F32R, tag="w2")
                            nc.scalar.dma_start(
                                w2t[:], w2r[:, ig * IG:(ig + 1) * IG,
                                            d * P:(d + 1) * P])
                            for ii in range(IG):
                                i = ig * IG + ii
                                for p0, pn in pieces:
                                    nc.tensor.matmul(
                                        pb[:, p0:p0 + pn], w2t[:, ii],
                                        g[:, i, p0:p0 + pn],
                                        start=(i == 0), stop=(i == IT - 1))
                        yt = ev.tile([P, MAXCH], F32, tag="ev")
                        if sc_bcast is not None:
                            # (f + b2) * w == f*w + b2*w; f*w is already in
                            # psum (g was pre-scaled), add b2[d]*wscale here.
                            bw = ev.tile([P, MAXCH], F32, tag="bw")
                            nc.vector.tensor_scalar_mul(
                                bw[:, :cn], sc_bcast[:, c0:c0 + cn],
                                b2_tile[:, d, None])
                            nc.vector.tensor_add(
                                out=yt[:, :cn], in0=pb[:, :cn], in1=bw[:, :cn])
                        else:
                            nc.vector.tensor_scalar_add(
                                yt[:, :cn], pb[:, :cn], b2_tile[:, d, None])
                        nc.gpsimd.dma_start(outr[:, d, c0:c0 + cn], yt[:, :cn])

            ffn(xgT_d, C, w1T_d, b1t, w2T_d, b2t, wscb, yeT_d)
            ffn(xsT_d, TS, sw1T_d, sb1t, sw2T_d, sb2t, None, ysT_d)

    nc.compile()
    return nc


_PROGRAM_CACHE = {}


def _get_program(C):
    if C not in _PROGRAM_CACHE:
        _PROGRAM_CACHE[C] = _build_program(C)
    return _PROGRAM_CACHE[C]


def _install_trace_shim():
    """Provide antenv.axon_hooks so run_bass_kernel_spmd(trace=True) can
    capture NTFF profiles under axon (mirrors trn_agent_boot.trn_boot)."""
    import contextlib
    import ctypes
    import sys
    import types

    if "antenv.axon_hooks" in sys.modules:
        return
    so_path = "/opt/axon/libaxon_pjrt.so"
    hook = None
    try:
        lib = ctypes.CDLL(so_path)
        if hasattr(lib, "axon_start_nrt_profile"):
            lib.axon_start_nrt_profile.argtypes = [
                ctypes.POINTER(ctypes.c_int64), ctypes.c_size_t]
            lib.axon_start_nrt_profile.restype = ctypes.c_int64
            lib.axon_stop_nrt_profile.argtypes = [ctypes.c_char_p]
            lib.axon_stop_nrt_profile.restype = ctypes.c_int64

            @contextlib.contextmanager
            def _hook(output_dir, device_ids):
                import jax
                jax.devices()
                if device_ids:
                    ids = (ctypes.c_int64 * len(device_ids))(*device_ids)
                    rc = lib.axon_start_nrt_profile(ids, len(device_ids))
                else:
                    rc = lib.axon_start_nrt_profile(None, 0)
                if rc != 0:
                    raise RuntimeError(f"axon_start_nrt_profile rc={rc}")
                try:
                    yield
                finally:
                    n = lib.axon_stop_nrt_profile(str(output_dir).encode())
                    print(f"ntff profile: {n} file(s) -> {output_dir}",
                          file=sys.stderr)

            hook = _hook
    except OSError:
        pass
    mod = types.ModuleType("antenv.axon_hooks")
    mod.get_axon_ntff_profile_hook = lambda: hook
    mod.set_axon_ntff_profile_hook = lambda h: None
    sys.modules["antenv.axon_hooks"] = mod
    import antenv
    antenv.axon_hooks = mod


def kernel(hidden_states, gate_w, e_w1, e_b1, e_w2, e_b2,
           s_w1, s_b1, s_w2, s_b2):
    global LAST_RESULTS
    from concourse.bass_utils import run_bass_kernel_spmd

    hidden_states = np.asarray(hidden_states, dtype=np.float32)
    gate_w = np.asarray(gate_w, dtype=np.float32)
    x = np.ascontiguousarray(hidden_states.reshape(T, D))

    # ---- gate: softmax + top-2 (host; 0.01% of total FLOPs) ----
    # float64 so the ranking agrees with any fp32 reference implementation
    # (fp32 impls deviate ~1e-8 from f64 in prob space; min top2/top3 margin
    # on this distribution is ~5e-7).
    logits = x.astype(np.float64) @ gate_w.T.astype(np.float64)
    m = logits.max(axis=-1, keepdims=True)
    p = np.exp(logits - m)
    p /= p.sum(axis=-1, keepdims=True)
    order = np.argsort(-p, axis=-1, kind="stable")
    top_idx = order[:, :TOP_K]                       # [T, 2]
    top_w = np.take_along_axis(p, top_idx, axis=-1)  # [T, 2]

    idx_e = [np.where((top_idx == e).any(axis=1))[0] for e in range(E)]
    w_e = []
    for e in range(E):
        sel = top_idx[idx_e[e]] == e
        w_e.append((top_w[idx_e[e]] * sel).sum(axis=1).astype(np.float32))

    counts = np.array([len(ix) for ix in idx_e])
    C = max(CHUNK, _round_up(int(counts.max()), CHUNK))

    nc = _get_program(C)

    xT = np.ascontiguousarray(x.T)  # [D, T]
    in_maps = []
    for e in range(E):
        n_e = counts[e]
        xgT = np.zeros((D, C), np.float32)
        xgT[:, :n_e] = xT[:, idx_e[e]]
        wsc = np.zeros((C,), np.float32)
        wsc[:n_e] = w_e[e]
        in_maps.append({
            "xgT": xgT,
            "wscb": np.broadcast_to(wsc, (P, C)).copy(),
            "w1T": np.ascontiguousarray(e_w1[e].T.astype(np.float32)),
            "b1": np.ascontiguousarray(e_b1[e].astype(np.float32)),
            "w2T": np.ascontiguousarray(e_w2[e].T.astype(np.float32)),
            "b2": np.ascontiguousarray(e_b2[e].astype(np.float32)),
            "xsT": np.ascontiguousarray(xT[:, e * TS:(e + 1) * TS]),
            "sw1T": np.ascontiguousarray(np.asarray(s_w1, np.float32).T),
            "sb1": np.ascontiguousarray(np.asarray(s_b1, np.float32)),
            "sw2T": np.ascontiguousarray(np.asarray(s_w2, np.float32).T),
            "sb2": np.ascontiguousarray(np.asarray(s_b2, np.float32)),
        })

    trace = os.environ.get("MOE_TRACE", "0") == "1"
    kwargs = {}
    if trace:
        _install_trace_shim()
        kwargs = dict(trace=True,
                      tmpdir=os.environ.get("MOE_TRACE_DIR") or None)
    res = run_bass_kernel_spmd(nc, in_maps, core_ids=list(range(E)), **kwargs)
    LAST_RESULTS = res

    y = np.empty((T, D), np.float32)
    for e in range(E):
        y[e * TS:(e + 1) * TS] = res.results[e]["ysT"].T
    for e in range(E):
        n_e = counts[e]
        y[idx_e[e]] += res.results[e]["yeT"][:, :n_e].T
    return y.reshape(B, S, D)


# revision 17
# speedup vs baseline: 1.0534x; 1.0120x over previous
"""MoE block kernel for Trainium2 (8 NeuronCores, Bass/Tile).

Strategy: expert-parallel with host-side top-2 dispatch.
  - Host computes the gate (softmax + top-2) in numpy (0.01% of FLOPs) and
    gathers each expert's tokens into a padded, transposed buffer.
  - Core e runs expert e's FFN over its gathered tokens (capacity C) plus
    the shared-expert FFN over a 1024-token slice, all in fp32r matmuls
    (full-rate PE, ~1.4e-4 matmul rel err) with fp32 PSUM accumulation.
  - Routing weights are folded into the activations after GELU (scale
    commutes with the second matmul); biases enter phase B as a K=1
    matmul row so outputs need no postprocessing.
  - Host scatters expert outputs back (y[idx_e] += ...) and stitches the
    shared slices.

Layouts (per core):
  phase A: g[i_tile] [128(I), tok]  = GELU(w1T_tile.T @ xT + b1) * wscale
  phase B: yT[d_tile] [128(D), tok] = sum_i w2T_tile.T @ g[i] + b2*wscale
"""

import os

import numpy as np

B, S, D, E, I = 2, 4096, 1024, 8, 4096
T = B * S
TOP_K = 2
TS = T // 8          # shared-expert tokens per core
CHUNK = 512          # tokens per weight-stream pass (one N=512 matmul per LDW)
P = 128

LAST_RESULTS = None  # BassKernelResults of the most recent run (set when tracing)


def _round_up(x, m):
    return ((x + m - 1) // m) * m


def _chunks(total):
    """Split total tokens into chunks of CHUNK then a 256-multiple tail."""
    out = []
    t = 0
    while total - t >= CHUNK:
        out.append((t, CHUNK))
        t += CHUNK
    if total - t:
        out.append((t, total - t))
        t = total
    return out


def _pieces(n):
    """Split a chunk into matmul moving-dim pieces of at most 512."""
    out = []
    t = 0
    while n - t > 512:
        out.append((t, 512))
        t += 512
    out.append((t, n - t))
    return out


def _build_program(C):
    import concourse.mybir as mybir
    import concourse.tile as tile
    from concourse import bacc

    F32, F32R = mybir.dt.float32, mybir.dt.float32r
    AF = mybir.ActivationFunctionType

    nc = bacc.Bacc("TRN2", target_bir_lowering=False, debug=False)

    xgT_d = nc.dram_tensor("xgT", [D, C], F32R, kind="ExternalInput")
    wscb_d = nc.dram_tensor("wscb", [P, C], F32R, kind="ExternalInput")
    w1T_d = nc.dram_tensor("w1T", [D, I], F32R, kind="ExternalInput")
    b1_d = nc.dram_tensor("b1", [I], F32, kind="ExternalInput")
    w2T_d = nc.dram_tensor("w2T", [I, D], F32R, kind="ExternalInput")
    b2_d = nc.dram_tensor("b2", [D], F32, kind="ExternalInput")
    xsT_d = nc.dram_tensor("xsT", [D, TS], F32R, kind="ExternalInput")
    sw1T_d = nc.dram_tensor("sw1T", [D, I], F32R, kind="ExternalInput")
    sb1_d = nc.dram_tensor("sb1", [I], F32, kind="ExternalInput")
    sw2T_d = nc.dram_tensor("sw2T", [I, D], F32R, kind="ExternalInput")
    sb2_d = nc.dram_tensor("sb2", [D], F32, kind="ExternalInput")
    yeT_d = nc.dram_tensor("yeT", [D, C], F32, kind="ExternalOutput")
    ysT_d = nc.dram_tensor("ysT", [D, TS], F32, kind="ExternalOutput")

    DT, IT = D // P, I // P        # 8 d-subtiles, 32 i-tiles
    IG = 8                         # i-tiles per w2 stream group
    MAXCH = CHUNK

    with tile.TileContext(nc) as tc:
        with (
            tc.tile_pool(name="const", bufs=1) as const,
            tc.tile_pool(name="act", bufs=1) as act,
            tc.tile_pool(name="xin", bufs=2) as xin,
            tc.tile_pool(name="w1p", bufs=8) as w1p,
            tc.tile_pool(name="w2p", bufs=8) as w2p,
            tc.tile_pool(name="ev", bufs=4) as ev,
            tc.tile_pool(name="psA", bufs=4, space="PSUM") as psA,
            tc.tile_pool(name="psB", bufs=4, space="PSUM") as psB,
        ):
            b1t = const.tile([P, IT], F32, tag="b1")
            nc.sync.dma_start(b1t[:], b1_d.ap().rearrange("(o p) -> p o", p=P))
            sb1t = const.tile([P, IT], F32, tag="sb1")
            nc.sync.dma_start(sb1t[:], sb1_d.ap().rearrange("(o p) -> p o", p=P))
            b2t = const.tile([P, DT], F32, tag="b2")
            nc.sync.dma_start(b2t[:], b2_d.ap().rearrange("(o p) -> p o", p=P))
            sb2t = const.tile([P, DT], F32, tag="sb2")
            nc.sync.dma_start(sb2t[:], sb2_d.ap().rearrange("(o p) -> p o", p=P))
            wscb = const.tile([P, C], F32R, tag="wscb")
            nc.gpsimd.dma_start(wscb[:], wscb_d.ap())

            g = act.tile([P, IT, MAXCH], F32R, tag="g")

            def ffn(xT_dram, ntok, w1T_dram, b1_tile, w2T_dram, b2_tile,
                    sc_bcast, outT_dram, first=False):
                w1r = w1T_dram.ap().rearrange("(o p) i -> p o i", p=P)
                w2r = w2T_dram.ap().rearrange("(o p) d -> p o d", p=P)
                xr = xT_dram.ap().rearrange("(o p) t -> p o t", p=P)
                outr = outT_dram.ap().rearrange("(o p) t -> p o t", p=P)
                for ci, (c0, cn) in enumerate(_chunks(ntok)):
                    pieces = _pieces(cn)
                    xt = xin.tile([P, DT, MAXCH], F32R, tag="x")
                    # per-k loads so the first matmul only waits for subtile 0
                    for k in range(DT):
                        nc.sync.dma_start(xt[:, k, :cn], xr[:, k, c0:c0 + cn])
                    # Phase A: g[i] = gelu(w1T_i.T @ x + b1_i) (* wscale)
                    # w1+x stream on the Sync HWDGE ring, w2 on the Scalar
                    # ring, outputs via SWDGE: three parallel DMA paths.
                    cold = first and ci == 0
                    w2_pref = {}
                    for i in range(IT):
                        w1t = w1p.tile([P, DT, P], F32R, tag="w1")
                        # on the cold start, the Scalar ring is idle: load the
                        # leading w1 tiles there so PE starts sooner
                        eng = nc.scalar if (cold and i < 4) else nc.sync
                        eng.dma_start(w1t[:], w1r[:, :, i * P:(i + 1) * P])
                        pa = psA.tile([P, MAXCH], F32, tag="psA")
                        for k in range(DT):
                            for p0, pn in pieces:
                                nc.tensor.matmul(
                                    pa[:, p0:p0 + pn], w1t[:, k],
                                    xt[:, k, p0:p0 + pn],
                                    start=(k == 0), stop=(k == DT - 1))
                        nc.scalar.activation(g[:, i, :cn], pa[:, :cn], AF.Gelu,
                                             bias=b1_tile[:, i, None])
                        if sc_bcast is not None:
                            nc.vector.tensor_mul(
                                out=g[:, i, :cn], in0=g[:, i, :cn],
                                in1=sc_bcast[:, c0:c0 + cn])
                        # hoist phase-B d=0 weight loads between GELUs so the
                        # A->B transition doesn't wait on the Scalar DMA FIFO
                        if i >= 8 and i % 4 == 0 and (i - 8) // 4 < IT // IG:
                            ig = (i - 8) // 4
                            w2t = w2p.tile([P, IG, P], F32R, tag="w2")
                            nc.scalar.dma_start(
                                w2t[:], w2r[:, ig * IG:(ig + 1) * IG, 0:P])
                            w2_pref[ig] = w2t
                    # Phase B: yT[d] = sum_i w2T_(i,d).T @ g[i] + b2_d * sc
                    for d in range(DT):
                        pb = psB.tile([P, MAXCH], F32, tag="psB")
                        for ig in range(IT // IG):
                            if d == 0:
                                w2t = w2_pref[ig]
                            else:
                                w2t = w2p.tile([P, IG, P], F32R, tag="w2")
                                nc.scalar.dma_start(
                                    w2t[:], w2r[:, ig * IG:(ig + 1) * IG,
                                                d * P:(d + 1) * P])
                            for ii in range(IG):
                                i = ig * IG + ii
                                for p0, pn in pieces:
                                    nc.tensor.matmul(
                                        pb[:, p0:p0 + pn], w2t[:, ii],
                                        g[:, i, p0:p0 + pn],
                                        start=(i == 0), stop=(i == IT - 1))
                        yt = ev.tile([P, MAXCH], F32, tag="ev")
                        if sc_bcast is not None:
                            # (f + b2) * w == f*w + b2*w; f*w is already in
                            # psum (g was pre-scaled), add b2[d]*wscale here.
                            bw = ev.tile([P, MAXCH], F32, tag="bw")
                            nc.vector.tensor_scalar_mul(
                                bw[:, :cn], sc_bcast[:, c0:c0 + cn],
                                b2_tile[:, d, None])
                            nc.vector.tensor_add(
                                out=yt[:, :cn], in0=pb[:, :cn], in1=bw[:, :cn])
                        else:
                            nc.vector.tensor_scalar_add(
                                yt[:, :cn], pb[:, :cn], b2_tile[:, d, None])
                        nc.gpsimd.dma_start(outr[:, d, c0:c0 + cn], yt[:, :cn])

            ffn(xgT_d, C, w1T_d, b1t, w2T_d, b2t, wscb, yeT_d, first=True)
            ffn(xsT_d, TS, sw1T_d, sb1t, sw2T_d, sb2t, None, ysT_d)

    nc.compile()
    return nc


_PROGRAM_CACHE = {}


def _get_program(C):
    if C not in _PROGRAM_CACHE:
        _PROGRAM_CACHE[C] = _build_program(C)
    return _PROGRAM_CACHE[C]


def _install_trace_shim():
    """Provide antenv.axon_hooks so run_bass_kernel_spmd(trace=True) can
    capture NTFF profiles under axon (mirrors trn_agent_boot.trn_boot)."""
    import contextlib
    import ctypes
    import sys
    import types

    if "antenv.axon_hooks" in sys.modules:
        return
    so_path = "/opt/axon/libaxon_pjrt.so"
    hook = None
    try:
        lib = ctypes.CDLL(so_path)
        if hasattr(lib, "axon_start_nrt_profile"):
            lib.axon_start_nrt_profile.argtypes = [
                ctypes.POINTER(ctypes.c_int64), ctypes.c_size_t]
            lib.axon_start_nrt_profile.restype = ctypes.c_int64
            lib.axon_stop_nrt_profile.argtypes = [ctypes.c_char_p]
            lib.axon_stop_nrt_profile.restype = ctypes.c_int64

            @contextlib.contextmanager
            def _hook(output_dir, device_ids):
                import jax
                jax.devices()
                if device_ids:
                    ids = (ctypes.c_int64 * len(device_ids))(*device_ids)
                    rc = lib.axon_start_nrt_profile(ids, len(device_ids))
                else:
                    rc = lib.axon_start_nrt_profile(None, 0)
                if rc != 0:
                    raise RuntimeError(f"axon_start_nrt_profile rc={rc}")
                try:
                    yield
                finally:
                    n = lib.axon_stop_nrt_profile(str(output_dir).encode())
                    print(f"ntff profile: {n} file(s) -> {output_dir}",
                          file=sys.stderr)

            hook = _hook
    except OSError:
        pass
    mod = types.ModuleType("antenv.axon_hooks")
    mod.get_axon_ntff_profile_hook = lambda: hook
    mod.set_axon_ntff_profile_hook = lambda h: None
    sys.modules["antenv.axon_hooks"] = mod
    import antenv
    antenv.axon_hooks = mod


def kernel(hidden_states, gate_w, e_w1, e_b1, e_w2, e_b2,
           s_w1, s_b1, s_w2, s_b2):
    global LAST_RESULTS
    from concourse.bass_utils import run_bass_kernel_spmd

    hidden_states = np.asarray(hidden_states, dtype=np.float32)
    gate_w = np.asarray(gate_w, dtype=np.float32)
    x = np.ascontiguousarray(hidden_states.reshape(T, D))

    # ---- gate: softmax + top-2 (host; 0.01% of total FLOPs) ----
    # float64 so the ranking agrees with any fp32 reference implementation
    # (fp32 impls deviate ~1e-8 from f64 in prob space; min top2/top3 margin
    # on this distribution is ~5e-7).
    logits = x.astype(np.float64) @ gate_w.T.astype(np.float64)
    m = logits.max(axis=-1, keepdims=True)
    p = np.exp(logits - m)
    p /= p.sum(axis=-1, keepdims=True)
    order = np.argsort(-p, axis=-1, kind="stable")
    top_idx = order[:, :TOP_K]                       # [T, 2]
    top_w = np.take_along_axis(p, top_idx, axis=-1)  # [T, 2]

    idx_e = [np.where((top_idx == e).any(axis=1))[0] for e in range(E)]
    w_e = []
    for e in range(E):
        sel = top_idx[idx_e[e]] == e
        w_e.append((top_w[idx_e[e]] * sel).sum(axis=1).astype(np.float32))

    counts = np.array([len(ix) for ix in idx_e])
    C = max(CHUNK, _round_up(int(counts.max()), CHUNK))

    nc = _get_program(C)

    xT = np.ascontiguousarray(x.T)  # [D, T]
    in_maps = []
    for e in range(E):
        n_e = counts[e]
        xgT = np.zeros((D, C), np.float32)
        xgT[:, :n_e] = xT[:, idx_e[e]]
        wsc = np.zeros((C,), np.float32)
        wsc[:n_e] = w_e[e]
        in_maps.append({
            "xgT": xgT,
            "wscb": np.broadcast_to(wsc, (P, C)).copy(),
            "w1T": np.ascontiguousarray(e_w1[e].T.astype(np.float32)),
            "b1": np.ascontiguousarray(e_b1[e].astype(np.float32)),
            "w2T": np.ascontiguousarray(e_w2[e].T.astype(np.float32)),
            "b2": np.ascontiguousarray(e_b2[e].astype(np.float32)),
            "xsT": np.ascontiguousarray(xT[:, e * TS:(e + 1) * TS]),
            "sw1T": np.ascontiguousarray(np.asarray(s_w1, np.float32).T),
            "sb1": np.ascontiguousarray(np.asarray(s_b1, np.float32)),
            "sw2T": np.ascontiguousarray(np.asarray(s_w2, np.float32).T),
            "sb2": np.ascontiguousarray(np.asarray(s_b2, np.float32)),
        })

    trace = os.environ.get("MOE_TRACE", "0") == "1"
    kwargs = {}
    if trace:
        _install_trace_shim()
        kwargs = dict(trace=True,
                      tmpdir=os.environ.get("MOE_TRACE_DIR") or None)
    res = run_bass_kernel_spmd(nc, in_maps, core_ids=list(range(E)), **kwargs)
    LAST_RESULTS = res

    y = np.empty((T, D), np.float32)
    for e in range(E):
        y[e * TS:(e + 1) * TS] = res.results[e]["ysT"].T
    for e in range(E):
        n_e = counts[e]
        y[idx_e[e]] += res.results[e]["yeT"][:, :n_e].T
    return y.reshape(B, S, D)


# revision 20
# speedup vs baseline: 1.0964x; 1.0408x over previous
"""MoE block kernel for Trainium2 (8 NeuronCores, Bass/Tile).

Strategy: expert-parallel with host-side top-2 dispatch.
  - Host computes the gate (softmax + top-2) in numpy (0.01% of FLOPs) and
    gathers each expert's tokens into a padded, transposed buffer.
  - Core e runs expert e's FFN over its gathered tokens (capacity C) plus
    the shared-expert FFN over a 1024-token slice, all in fp32r matmuls
    (full-rate PE, ~1.4e-4 matmul rel err) with fp32 PSUM accumulation.
  - Routing weights are folded into the activations after GELU (scale
    commutes with the second matmul); biases enter phase B as a K=1
    matmul row so outputs need no postprocessing.
  - Host scatters expert outputs back (y[idx_e] += ...) and stitches the
    shared slices.

Layouts (per core):
  phase A: g[i_tile] [128(I), tok]  = GELU(w1T_tile.T @ xT + b1) * wscale
  phase B: yT[d_tile] [128(D), tok] = sum_i w2T_tile.T @ g[i] + b2*wscale
"""

import os

import numpy as np

B, S, D, E, I = 2, 4096, 1024, 8, 4096
T = B * S
TOP_K = 2
TS = T // 8          # shared-expert tokens per core
CHUNK = 512          # tokens per weight-stream pass (one N=512 matmul per LDW)
P = 128

LAST_RESULTS = None  # BassKernelResults of the most recent run (set when tracing)


def _round_up(x, m):
    return ((x + m - 1) // m) * m


def _chunks(total):
    """Split total tokens into chunks of CHUNK then a 256-multiple tail."""
    out = []
    t = 0
    while total - t >= CHUNK:
        out.append((t, CHUNK))
        t += CHUNK
    if total - t:
        out.append((t, total - t))
        t = total
    return out


def _pieces(n):
    """Split a chunk into matmul moving-dim pieces of at most 512."""
    out = []
    t = 0
    while n - t > 512:
        out.append((t, 512))
        t += 512
    out.append((t, n - t))
    return out


def _build_program(C):
    import concourse.mybir as mybir
    import concourse.tile as tile
    from concourse import bacc

    F32, F32R = mybir.dt.float32, mybir.dt.float32r
    AF = mybir.ActivationFunctionType

    nc = bacc.Bacc("TRN2", target_bir_lowering=False, debug=False)

    xgT_d = nc.dram_tensor("xgT", [D, C], F32R, kind="ExternalInput")
    wscb_d = nc.dram_tensor("wscb", [P, C], F32R, kind="ExternalInput")
    w1T_d = nc.dram_tensor("w1T", [D, I], F32R, kind="ExternalInput")
    b1_d = nc.dram_tensor("b1", [I], F32, kind="ExternalInput")
    w2T_d = nc.dram_tensor("w2T", [I, D], F32R, kind="ExternalInput")
    b2_d = nc.dram_tensor("b2", [D], F32, kind="ExternalInput")
    xsT_d = nc.dram_tensor("xsT", [D, TS], F32R, kind="ExternalInput")
    sw1T_d = nc.dram_tensor("sw1T", [D, I], F32R, kind="ExternalInput")
    sb1_d = nc.dram_tensor("sb1", [I], F32, kind="ExternalInput")
    sw2T_d = nc.dram_tensor("sw2T", [I, D], F32R, kind="ExternalInput")
    sb2_d = nc.dram_tensor("sb2", [D], F32, kind="ExternalInput")
    yeT_d = nc.dram_tensor("yeT", [D, C], F32, kind="ExternalOutput")
    ysT_d = nc.dram_tensor("ysT", [D, TS], F32, kind="ExternalOutput")

    DT, IT = D // P, I // P        # 8 d-subtiles, 32 i-tiles
    IG = 16                        # i-tiles per w2 stream group (1MB DMAs)
    MAXCH = CHUNK

    with tile.TileContext(nc) as tc:
        with (
            tc.tile_pool(name="const", bufs=1) as const,
            tc.tile_pool(name="act", bufs=1) as act,
            tc.tile_pool(name="xin", bufs=1) as xin,
            tc.tile_pool(name="w1p", bufs=6) as w1p,
            tc.tile_pool(name="w2p", bufs=4) as w2p,
            tc.tile_pool(name="ev", bufs=3) as ev,
            tc.tile_pool(name="psA", bufs=4, space="PSUM") as psA,
            tc.tile_pool(name="psB", bufs=4, space="PSUM") as psB,
        ):
            b1t = const.tile([P, IT], F32, tag="b1")
            nc.sync.dma_start(b1t[:], b1_d.ap().rearrange("(o p) -> p o", p=P))
            sb1t = const.tile([P, IT], F32, tag="sb1")
            nc.sync.dma_start(sb1t[:], sb1_d.ap().rearrange("(o p) -> p o", p=P))
            b2t = const.tile([P, DT], F32, tag="b2")
            nc.sync.dma_start(b2t[:], b2_d.ap().rearrange("(o p) -> p o", p=P))
            sb2t = const.tile([P, DT], F32, tag="sb2")
            nc.sync.dma_start(sb2t[:], sb2_d.ap().rearrange("(o p) -> p o", p=P))
            wscb = const.tile([P, C], F32R, tag="wscb")
            nc.gpsimd.dma_start(wscb[:], wscb_d.ap())

            g = act.tile([P, IT, MAXCH], F32R, tag="g")

            def ffn(xT_dram, ntok, w1T_dram, b1_tile, w2T_dram, b2_tile,
                    sc_bcast, outT_dram, first=False):
                w1r = w1T_dram.ap().rearrange("(o p) i -> p o i", p=P)
                w2r = w2T_dram.ap().rearrange("(o p) d -> p o d", p=P)
                xr = xT_dram.ap().rearrange("(o p) t -> p o t", p=P)
                outr = outT_dram.ap().rearrange("(o p) t -> p o t", p=P)
                for ci, (c0, cn) in enumerate(_chunks(ntok)):
                    pieces = _pieces(cn)
                    xt = xin.tile([P, DT, MAXCH], F32R, tag="x")
                    # per-k SWDGE loads so the first matmul only waits for
                    # subtile 0 and the HWDGE rings stay weight-only
                    for k in range(DT):
                        nc.gpsimd.dma_start(xt[:, k, :cn], xr[:, k, c0:c0 + cn])
                    # Phase A: g[i] = gelu(w1T_i.T @ x + b1_i) (* wscale)
                    # w1 streams on the Sync HWDGE ring (1MB tiles = 2
                    # i-tiles), w2 on the Scalar ring, x/outputs via SWDGE.
                    cold = first and ci == 0
                    w2_pref = {}
                    W1G = 2              # i-tiles per w1 DMA
                    for si in range(IT // W1G):
                        w1t = w1p.tile([P, DT, W1G * P], F32R, tag="w1")
                        # on the cold start, the Scalar ring is idle: load the
                        # leading w1 tiles there so PE starts sooner
                        eng = nc.scalar if (cold and si < 3) else nc.sync
                        eng.dma_start(
                            w1t[:], w1r[:, :, si * W1G * P:(si + 1) * W1G * P])
                        for sub in range(W1G):
                            i = si * W1G + sub
                            pa = psA.tile([P, MAXCH], F32, tag="psA")
                            for k in range(DT):
                                for p0, pn in pieces:
                                    nc.tensor.matmul(
                                        pa[:, p0:p0 + pn],
                                        w1t[:, k, sub * P:(sub + 1) * P],
                                        xt[:, k, p0:p0 + pn],
                                        start=(k == 0), stop=(k == DT - 1))
                            nc.scalar.activation(g[:, i, :cn], pa[:, :cn],
                                                 AF.Gelu,
                                                 bias=b1_tile[:, i, None])
                            if sc_bcast is not None:
                                nc.vector.tensor_mul(
                                    out=g[:, i, :cn], in0=g[:, i, :cn],
                                    in1=sc_bcast[:, c0:c0 + cn])
                            # hoist phase-B d=0 weight loads between GELUs so
                            # the A->B transition doesn't wait on Scalar DMAs
                            if i in (12, 20):
                                ig = (i - 12) // 8
                                w2t = w2p.tile([P, IG, P], F32R, tag="w2")
                                nc.scalar.dma_start(
                                    w2t[:],
                                    w2r[:, ig * IG:(ig + 1) * IG, 0:P])
                                w2_pref[ig] = w2t
                    # Phase B: yT[d] = sum_i w2T_(i,d).T @ g[i] + b2_d * sc
                    for d in range(DT):
                        pb = psB.tile([P, MAXCH], F32, tag="psB")
                        for ig in range(IT // IG):
                            if d == 0:
                                w2t = w2_pref[ig]
                            else:
                                w2t = w2p.tile([P, IG, P], F32R, tag="w2")
                                nc.scalar.dma_start(
                                    w2t[:], w2r[:, ig * IG:(ig + 1) * IG,
                                                d * P:(d + 1) * P])
                            for ii in range(IG):
                                i = ig * IG + ii
                                for p0, pn in pieces:
                                    nc.tensor.matmul(
                                        pb[:, p0:p0 + pn], w2t[:, ii],
                                        g[:, i, p0:p0 + pn],
                                        start=(i == 0), stop=(i == IT - 1))
                        yt = ev.tile([P, MAXCH], F32, tag="ev")
                        if sc_bcast is not None:
                            # (f + b2) * w == f*w + b2*w; f*w is already in
                            # psum (g was pre-scaled), add b2[d]*wscale here.
                            bw = ev.tile([P, MAXCH], F32, tag="bw")
                            nc.vector.tensor_scalar_mul(
                                bw[:, :cn], sc_bcast[:, c0:c0 + cn],
                                b2_tile[:, d, None])
                            nc.vector.tensor_add(
                                out=yt[:, :cn], in0=pb[:, :cn], in1=bw[:, :cn])
                        else:
                            nc.vector.tensor_scalar_add(
                                yt[:, :cn], pb[:, :cn], b2_tile[:, d, None])
                        nc.gpsimd.dma_start(outr[:, d, c0:c0 + cn], yt[:, :cn])

            ffn(xgT_d, C, w1T_d, b1t, w2T_d, b2t, wscb, yeT_d, first=True)
            ffn(xsT_d, TS, sw1T_d, sb1t, sw2T_d, sb2t, None, ysT_d)

    nc.compile()
    return nc


_PROGRAM_CACHE = {}


def _get_program(C):
    if C not in _PROGRAM_CACHE:
        _PROGRAM_CACHE[C] = _build_program(C)
    return _PROGRAM_CACHE[C]


def _install_trace_shim():
    """Provide antenv.axon_hooks so run_bass_kernel_spmd(trace=True) can
    capture NTFF profiles under axon (mirrors trn_agent_boot.trn_boot)."""
    import contextlib
    import ctypes
    import sys
    import types

    if "antenv.axon_hooks" in sys.modules:
        return
    so_path = "/opt/axon/libaxon_pjrt.so"
    hook = None
    try:
        lib = ctypes.CDLL(so_path)
        if hasattr(lib, "axon_start_nrt_profile"):
            lib.axon_start_nrt_profile.argtypes = [
                ctypes.POINTER(ctypes.c_int64), ctypes.c_size_t]
            lib.axon_start_nrt_profile.restype = ctypes.c_int64
            lib.axon_stop_nrt_profile.argtypes = [ctypes.c_char_p]
            lib.axon_stop_nrt_profile.restype = ctypes.c_int64

            @contextlib.contextmanager
            def _hook(output_dir, device_ids):
                import jax
                jax.devices()
                if device_ids:
                    ids = (ctypes.c_int64 * len(device_ids))(*device_ids)
                    rc = lib.axon_start_nrt_profile(ids, len(device_ids))
                else:
                    rc = lib.axon_start_nrt_profile(None, 0)
                if rc != 0:
                    raise RuntimeError(f"axon_start_nrt_profile rc={rc}")
                try:
                    yield
                finally:
                    n = lib.axon_stop_nrt_profile(str(output_dir).encode())
                    print(f"ntff profile: {n} file(s) -> {output_dir}",
                          file=sys.stderr)

            hook = _hook
    except OSError:
        pass
    mod = types.ModuleType("antenv.axon_hooks")
    mod.get_axon_ntff_profile_hook = lambda: hook
    mod.set_axon_ntff_profile_hook = lambda h: None
    sys.modules["antenv.axon_hooks"] = mod
    import antenv
    antenv.axon_hooks = mod


def kernel(hidden_states, gate_w, e_w1, e_b1, e_w2, e_b2,
           s_w1, s_b1, s_w2, s_b2):
    global LAST_RESULTS
    from concourse.bass_utils import run_bass_kernel_spmd

    hidden_states = np.asarray(hidden_states, dtype=np.float32)
    gate_w = np.asarray(gate_w, dtype=np.float32)
    x = np.ascontiguousarray(hidden_states.reshape(T, D))

    # ---- gate: softmax + top-2 (host; 0.01% of total FLOPs) ----
    # float64 so the ranking agrees with any fp32 reference implementation
    # (fp32 impls deviate ~1e-8 from f64 in prob space; min top2/top3 margin
    # on this distribution is ~5e-7).
    logits = x.astype(np.float64) @ gate_w.T.astype(np.float64)
    m = logits.max(axis=-1, keepdims=True)
    p = np.exp(logits - m)
    p /= p.sum(axis=-1, keepdims=True)
    order = np.argsort(-p, axis=-1, kind="stable")
    top_idx = order[:, :TOP_K]                       # [T, 2]
    top_w = np.take_along_axis(p, top_idx, axis=-1)  # [T, 2]

    idx_e = [np.where((top_idx == e).any(axis=1))[0] for e in range(E)]
    w_e = []
    for e in range(E):
        sel = top_idx[idx_e[e]] == e
        w_e.append((top_w[idx_e[e]] * sel).sum(axis=1).astype(np.float32))

    counts = np.array([len(ix) for ix in idx_e])
    C = max(CHUNK, _round_up(int(counts.max()), CHUNK))

    nc = _get_program(C)

    xT = np.ascontiguousarray(x.T)  # [D, T]
    in_maps = []
    for e in range(E):
        n_e = counts[e]
        xgT = np.zeros((D, C), np.float32)
        xgT[:, :n_e] = xT[:, idx_e[e]]
        wsc = np.zeros((C,), np.float32)
        wsc[:n_e] = w_e[e]
        in_maps.append({
            "xgT": xgT,
            "wscb": np.broadcast_to(wsc, (P, C)).copy(),
            "w1T": np.ascontiguousarray(e_w1[e].T.astype(np.float32)),
            "b1": np.ascontiguousarray(e_b1[e].astype(np.float32)),
            "w2T": np.ascontiguousarray(e_w2[e].T.astype(np.float32)),
            "b2": np.ascontiguousarray(e_b2[e].astype(np.float32)),
            "xsT": np.ascontiguousarray(xT[:, e * TS:(e + 1) * TS]),
            "sw1T": np.ascontiguousarray(np.asarray(s_w1, np.float32).T),
            "sb1": np.ascontiguousarray(np.asarray(s_b1, np.float32)),
            "sw2T": np.ascontiguousarray(np.asarray(s_w2, np.float32).T),
            "sb2": np.ascontiguousarray(np.asarray(s_b2, np.float32)),
        })

    trace = os.environ.get("MOE_TRACE", "0") == "1"
    kwargs = {}
    if trace:
        _install_trace_shim()
        kwargs = dict(trace=True,
                      tmpdir=os.environ.get("MOE_TRACE_DIR") or None)
    res = run_bass_kernel_spmd(nc, in_maps, core_ids=list(range(E)), **kwargs)
    LAST_RESULTS = res

    y = np.empty((T, D), np.float32)
    for e in range(E):
        y[e * TS:(e + 1) * TS] = res.results[e]["ysT"].T
    for e in range(E):
        n_e = counts[e]
        y[idx_e[e]] += res.results[e]["yeT"][:, :n_e].T
    return y.reshape(B, S, D)
